# revision 43
# baseline (speedup 1.0000x reference)
"""Trainium2 Bass kernel for BatchedACE (LSH-softmax linear attention).

Math (per fused sequence n of N = M*B*H = 32):
  probs(X)[t, l, r] = softmax_r( tanh(X @ planes)/sqrt(dk) @ protos )
  A = cumsum_t(probsK)                      [T, L, R]
  S_t = cumsum_t(probsK x V outer)          [L, R, dk]
  out[t] = sum_{l,r} probsQ[t,l,r] * S_t[l,r,:] / (A[t,l,r] + 1e-6)

Layout: L*R = 128 = partition dim; everything in [lr, t]. Chunked linear
attention: per 128-chunk, out[t',d] = (mask o (P^T Qn))^T V + Qn^T S.

Optimizations over the v0 kernel:
  * K-first phase skew with per-engine emission ordering: K's probs
    chain completes first so the DVE scan/normalize stream starts early
    while Q's chain fills PE/ACT.
  * ACT converts the softmax-recip broadcast (f32 PSUM) to bf16 SBUF so
    the two big normalize muls run in the 2x 16-bit DVE mode.
  * Output matmuls produce [t',d] (free dim 64, half the PE stream).
  * 2-seq-packed projection (128-deep contraction), per-w-half tanh.
  * Inputs merged into 3 DMAs (tiny weights DMA first + PE warmup
    matmuls while kt streams); v/consts DMA issued late; bf16 output.
  * Pool (GpSimd) does the SBUF->SBUF bf16 conversions (it cannot touch
    PSUM); scans/divides are DVE-only on this ISA.

Sharding: N=32 sequences split 4-per-core across 8 NeuronCores.
"""
import numpy as np
import ml_dtypes
from contextlib import ExitStack

import concourse.bass as bass
import concourse.tile as tile
from concourse import bacc, mybir
from concourse.bass_utils import run_bass_kernel_spmd

BF16 = ml_dtypes.bfloat16
BF = mybir.dt.bfloat16
F32 = mybir.dt.float32
Alu = mybir.AluOpType
Act = mybir.ActivationFunctionType

M_ENS, B_SZ, T_LEN, H_HEADS, D_K = 2, 2, 512, 8, 64
K_BITS, L_TABLES, R_CORNERS = 4, 8, 16
N_TOTAL = M_ENS * B_SZ * H_HEADS          # 32
NCORES = 8
SEQ = N_TOTAL // NCORES                   # 4 sequences per core
CH = 128                                  # chunk length (partition dim)
NCH = T_LEN // CH                         # 4 chunks
LR = L_TABLES * R_CORNERS                 # 128
LK = L_TABLES * K_BITS                    # 32

_CACHE = {}


def _build_module(n_iters=1):
    """n_iters>1 wraps the body in a hardware For_i loop (timing builds)."""
    nc = bacc.Bacc("TRN2", target_bir_lowering=False, debug=False,
                   num_devices=NCORES)

    # per-core inputs, merged into two DMAs (seq sigma -> (u=sigma%2, w=sigma//2)):
    #   ain = [kt (1024) | planes2 (64) | w4 (128)]              [128, 1216]
    #   bin = [qt (1024) | v (1024) | mask4 (512) | bones4 (128)
    #          | ident (128) | ones32 (32)]                      [128, 2848]
    ain_d = nc.dram_tensor("ain", [128, 2 * T_LEN + 192], BF,
                           kind="ExternalInput").ap()
    bin_d = nc.dram_tensor("bin", [128, 2 * T_LEN + SEQ * NCH * D_K + 800],
                           BF, kind="ExternalInput").ap()
    out_d = nc.dram_tensor("out_t", [SEQ, T_LEN, D_K], BF, kind="ExternalOutput").ap()

    with tile.TileContext(nc) as tc:
        with ExitStack() as ctx:
            cp = ctx.enter_context(tc.tile_pool(name="consts", bufs=1))
            sp = ctx.enter_context(tc.tile_pool(name="sb", bufs=1))
            lp = ctx.enter_context(tc.tile_pool(name="loop", bufs=5))
            pbig = ctx.enter_context(tc.tile_pool(name="pbig", bufs=2, space="PSUM"))
            psm = ctx.enter_context(tc.tile_pool(name="psm", bufs=2, space="PSUM"))
            pbc = ctx.enter_context(tc.tile_pool(name="pbc", bufs=1, space="PSUM"))
            if n_iters > 1:
                ctx.enter_context(tc.For_i(0, n_iters, 1,
                                           hint_engines=(mybir.EngineType.PE,)))

            ain_sb = sp.tile([128, 2 * T_LEN + 192], BF)
            nc.sync.dma_start(ain_sb[:, 2 * T_LEN:], ain_d[:, 2 * T_LEN:])
            nc.sync.dma_start(ain_sb[:, 0:2 * T_LEN], ain_d[:, 0:2 * T_LEN])
            bin_sb = sp.tile([128, 2 * T_LEN + SEQ * NCH * D_K + 800], BF)
            nc.scalar.dma_start(bin_sb[:, 0:2 * T_LEN], bin_d[:, 0:2 * T_LEN])

            kt_sb = ain_sb[:, 0:2 * T_LEN]
            planes2_sb = ain_sb[:, 2 * T_LEN:2 * T_LEN + 64]
            w4_sb = ain_sb[:, 2 * T_LEN + 64:2 * T_LEN + 192]
            qt_sb = bin_sb[:, 0:2 * T_LEN]
            v_sb = bin_sb[:, 2 * T_LEN:2 * T_LEN + SEQ * NCH * D_K]
            cb = 2 * T_LEN + SEQ * NCH * D_K
            mask4_sb = bin_sb[:, cb:cb + 512]
            bones4_sb = bin_sb[:, cb + 512:cb + 640]
            ident_sb = bin_sb[:, cb + 640:cb + 768]
            ones32_sb = bin_sb[:, cb + 768:cb + 800]

            def S(s):
                return slice(T_LEN * s, T_LEN * (s + 1))

            def tsl(s, c):
                return slice(T_LEN * s + CH * c, T_LEN * s + CH * (c + 1))

            def vsl(s, c):
                return slice(D_K * (s * NCH + c), D_K * (s * NCH + c + 1))

            # PE warmup: ramp the tensor engine while kt streams in
            warm_ps = pbc.tile([64, 2 * T_LEN], F32, tag="bc", name="warm")
            for i in range(16):
                nc.tensor.matmul(warm_ps[:, 64 * i:64 * (i + 1)],
                                 planes2_sb, planes2_sb,
                                 start=True, stop=True, tile_position=(0, 0))

            # ---- probs pipelines, K first so its chain starts earliest ----
            xt = {"k": kt_sb, "q": qt_sb}
            tanh_t = {}

            def emit_proj(x):
                proj_ps = pbig.tile([64, 2 * T_LEN], F32, tag="w", name=f"pj{x}")
                for w in range(2):
                    nc.tensor.matmul(
                        proj_ps[:, T_LEN * w:T_LEN * (w + 1)],
                        planes2_sb, xt[x][:, T_LEN * w:T_LEN * (w + 1)],
                        start=True, stop=True, tile_position=(0, 0))
                tanh_t[x] = lp.tile([64, 2 * T_LEN], BF, tag=f"tanh{x}",
                                    name=f"tanh{x}")
                return proj_ps

            def emit_tanh(x, proj_ps, w):
                cols = slice(T_LEN * w, T_LEN * (w + 1))
                nc.scalar.activation(tanh_t[x][:, cols], proj_ps[:, cols],
                                     Act.Tanh)

            def tanh_slice(x, sig):
                u, w = sig % 2, sig // 2
                return (tanh_t[x][32 * u:32 * u + 32,
                                  T_LEN * w:T_LEN * (w + 1)], 32 * u)

            # K-first phase skew: K's logits/exp/sums/recip/bc/mul complete
            # first so the scans (DVE) start early; Q's chain fills PE/ACT
            # while DVE works.
            e_t = {"k": sp.tile([128, SEQ * T_LEN], BF, name="ek"),
                   "q": sp.tile([128, SEQ * T_LEN], BF, name="eq")}
            pt_sb = sp.tile([128, SEQ * T_LEN], BF, name="ptk")
            qe_sb = sp.tile([128, SEQ * T_LEN], BF, name="qe")
            dst = {"k": pt_sb, "q": qe_sb}
            a_sb = sp.tile([128, SEQ * T_LEN], F32, name="a")
            ra_sb = sp.tile([128, SEQ * T_LEN], F32, name="ra")
            rab_sb = sp.tile([128, SEQ * T_LEN], BF, name="rab")
            qn_sb = sp.tile([128, SEQ * T_LEN], BF, name="qn")

            def esl(x, sig):
                return e_t[x][:, T_LEN * sig:T_LEN * (sig + 1)]

            def emit_lg(x, h):
                t_ = pbig.tile([128, 2 * T_LEN], F32, tag="w", name=f"lg{x}{h}")
                for j in range(2):
                    sig = 2 * h + j
                    tsb, r0 = tanh_slice(x, sig)
                    nc.tensor.matmul(t_[:, T_LEN * j:T_LEN * (j + 1)],
                                     w4_sb[r0:r0 + 32, :], tsb,
                                     start=True, stop=True,
                                     tile_position=(r0, 0))
                return t_

            def emit_exp(x, h, t_):
                nc.scalar.activation(
                    e_t[x][:, 2 * T_LEN * h:2 * T_LEN * (h + 1)],
                    t_[:], Act.Exp)

            def emit_sums(x, sums_ps, h):
                for j in range(2):
                    sig = 2 * h + j
                    nc.tensor.matmul(sums_ps[32 * sig:32 * sig + 32, :],
                                     ones32_sb, esl(x, sig),
                                     start=True, stop=True,
                                     tile_position=(0, 32 * sig))

            def emit_recip(x, sums_ps):
                recip_f = lp.tile([128, T_LEN], F32, tag=f"recf{x}",
                                  name=f"recf{x}")
                nc.vector.reciprocal_approx_fast(recip_f[:], sums_ps[:])
                recip_b = lp.tile([128, T_LEN], BF, tag=f"recb{x}",
                                  name=f"recb{x}")
                nc.gpsimd.tensor_copy(recip_b[:], recip_f[:])
                return recip_b

            bcb_sb = sp.tile([128, 2 * SEQ * T_LEN], BF, name="bcb")

            def emit_norm(x, recip_b, h):
                # bc matmul (f32 psum) -> ACT copy to bf16 SBUF -> the big
                # normalize mul runs in the 2x 16-bit DVE mode.
                b = pbc.tile([128, 2 * T_LEN], F32, tag="bc", name=f"bc{x}{h}")
                for j in range(2):
                    sig = 2 * h + j
                    nc.tensor.matmul(b[:, T_LEN * j:T_LEN * (j + 1)],
                                     bones4_sb[32 * sig:32 * sig + 8, :],
                                     recip_b[32 * sig:32 * sig + 8, :],
                                     start=True, stop=True,
                                     tile_position=(32 * sig, 0))
                xo = 0 if x == "k" else SEQ * T_LEN
                cols = slice(2 * T_LEN * h, 2 * T_LEN * (h + 1))
                bcols = slice(xo + 2 * T_LEN * h, xo + 2 * T_LEN * (h + 1))
                nc.scalar.copy(bcb_sb[:, bcols], b[:])
                nc.vector.tensor_mul(dst[x][:, cols], e_t[x][:, cols],
                                     bcb_sb[:, bcols])

            def emit_scan(sig, eng=None):
                eng = eng or nc.vector
                eng.tensor_tensor_scan(a_sb[:, S(sig)], pt_sb[:, S(sig)],
                                       pt_sb[:, S(sig)], 0.0,
                                       Alu.add, Alu.bypass)

            def emit_qdiv(sig, eng=None):
                nc.vector.reciprocal_approx_fast(ra_sb[:, S(sig)],
                                                 a_sb[:, S(sig)])
                nc.gpsimd.tensor_copy(rab_sb[:, S(sig)], ra_sb[:, S(sig)])
                nc.vector.tensor_mul(qn_sb[:, S(sig)], qe_sb[:, S(sig)],
                                     rab_sb[:, S(sig)])

            # attention building blocks
            pn_sb, gm_sb, s_tiles = {}, {}, []

            def emit_trans(c):
                tr_ps = psm.tile([CH, SEQ * CH], BF, tag="s", name=f"tr{c}")
                for sig in range(SEQ):
                    nc.tensor.transpose(tr_ps[:, CH * sig:CH * (sig + 1)],
                                        pt_sb[:, tsl(sig, c)], ident_sb[:])
                pn_sb[c] = lp.tile([CH, SEQ * CH], BF, tag="pn", name=f"pn{c}")
                nc.scalar.copy(pn_sb[c][:], tr_ps[:])

            def emit_ds(c):
                ds_ps = psm.tile([LR, SEQ * D_K], F32, tag="s", name=f"ds{c}")
                for sig in range(SEQ):
                    nc.tensor.matmul(ds_ps[:, D_K * sig:D_K * (sig + 1)],
                                     pn_sb[c][:, CH * sig:CH * (sig + 1)],
                                     v_sb[:, vsl(sig, c)],
                                     start=True, stop=True)
                s_new = lp.tile([LR, SEQ * D_K], BF, tag=f"st{c}", name=f"st{c}")
                if not s_tiles:
                    nc.scalar.copy(s_new[:], ds_ps[:])
                else:
                    nc.vector.tensor_add(s_new[:], ds_ps[:], s_tiles[-1][:])
                s_tiles.append(s_new)

            def emit_gt(c):
                gt_ps = psm.tile([CH, SEQ * CH], F32, tag="s", name=f"gt{c}")
                for sig in range(SEQ):
                    nc.tensor.matmul(gt_ps[:, CH * sig:CH * (sig + 1)],
                                     pt_sb[:, tsl(sig, c)],
                                     qn_sb[:, tsl(sig, c)],
                                     start=True, stop=True)
                gm_sb[c] = lp.tile([CH, SEQ * CH], BF, tag="gm", name=f"gm{c}")
                nc.vector.tensor_mul(gm_sb[c][:], gt_ps[:], mask4_sb[:])

            pjk = emit_proj("k")
            emit_tanh("k", pjk, 0)
            lgk0 = emit_lg("k", 0)
            emit_exp("k", 0, lgk0)
            emit_tanh("k", pjk, 1)
            lgk1 = emit_lg("k", 1)
            emit_exp("k", 1, lgk1)
            pjq = emit_proj("q")
            nc.sync.dma_start(bin_sb[:, 2 * T_LEN:], bin_d[:, 2 * T_LEN:])
            emit_tanh("q", pjq, 0)
            sums_k = psm.tile([128, T_LEN], F32, tag="s", name="sumsk")
            emit_sums("k", sums_k, 0)
            emit_sums("k", sums_k, 1)
            rbk = emit_recip("k", sums_k)
            emit_norm("k", rbk, 0)
            emit_scan(0)
            emit_norm("k", rbk, 1)
            emit_scan(1)
            lgq0 = emit_lg("q", 0)
            emit_exp("q", 0, lgq0)
            emit_tanh("q", pjq, 1)
            lgq1 = emit_lg("q", 1)
            emit_exp("q", 1, lgq1)
            emit_scan(2)
            sums_q = psm.tile([128, T_LEN], F32, tag="s", name="sumsq")
            emit_sums("q", sums_q, 0)
            emit_sums("q", sums_q, 1)
            emit_scan(3)
            rbq = emit_recip("q", sums_q)
            emit_trans(0)
            emit_norm("q", rbq, 0)
            emit_qdiv(0)
            emit_qdiv(1)
            emit_trans(1)
            emit_norm("q", rbq, 1)
            emit_qdiv(2)
            emit_qdiv(3)
            emit_trans(2)
            emit_ds(0)
            emit_ds(1)
            emit_ds(2)
            for c in range(NCH):
                emit_gt(c)

            # out[t', d] = gm_c^T V_c + Qn_c^T S_{c-1}
            for c in range(NCH):
                o_ps = psm.tile([CH, SEQ * D_K], F32, tag="s", name=f"o{c}")
                for sig in range(SEQ):
                    nc.tensor.matmul(o_ps[:, D_K * sig:D_K * (sig + 1)],
                                     gm_sb[c][:, CH * sig:CH * (sig + 1)],
                                     v_sb[:, vsl(sig, c)],
                                     start=True, stop=(c == 0))
                    if c > 0:
                        nc.tensor.matmul(o_ps[:, D_K * sig:D_K * (sig + 1)],
                                         qn_sb[:, tsl(sig, c)],
                                         s_tiles[c - 1][:, D_K * sig:D_K * (sig + 1)],
                                         start=False, stop=True)
                o_sb = lp.tile([CH, SEQ * D_K], BF, tag="osb", name=f"osb{c}")
                eng = (nc.scalar.copy, nc.vector.tensor_copy,
                       nc.scalar.copy, nc.scalar.copy)[c]
                eng(o_sb[:], o_ps[:])
                nc.sync.dma_start(
                    out_d[:, CH * c:CH * (c + 1), :].rearrange("s t d -> t s d"),
                    o_sb[:].rearrange("t (s d) -> t s d", s=SEQ))

    nc.compile()
    return nc


def _host_prep(Khf, Vhf, Qhf, planes_T, protos_T):
    """Fold + transpose + quantize inputs; build per-core in_maps."""
    Khf = np.asarray(Khf, dtype=np.float32)
    Vhf = np.asarray(Vhf, dtype=np.float32)
    Qhf = np.asarray(Qhf, dtype=np.float32)
    planes_T = np.asarray(planes_T, dtype=np.float32)
    protos_T = np.asarray(protos_T, dtype=np.float32)
    scale = np.sqrt(np.float32(D_K))

    def fold(x):
        return np.transpose(x, (0, 1, 3, 2, 4)).reshape(N_TOTAL, T_LEN, D_K)

    K2, Q2, V2 = fold(Khf), fold(Qhf), fold(Vhf)
    KT = np.ascontiguousarray(np.transpose(K2, (0, 2, 1))).astype(BF16)  # [N, dk, T]
    QT = np.ascontiguousarray(np.transpose(Q2, (0, 2, 1))).astype(BF16)
    V4 = V2.reshape(N_TOTAL, NCH, CH, D_K)

    # planes2: block-diag [128, 64]; w4: per-32-row table weights [128, 128]
    planes2 = np.zeros((128, 64), dtype=BF16)
    pT = planes_T.astype(BF16)                        # [64, 32]
    planes2[0:64, 0:32] = pT
    planes2[64:128, 32:64] = pT
    wblk = np.zeros((LK, LR), dtype=np.float32)
    for l in range(L_TABLES):
        wblk[l * K_BITS:(l + 1) * K_BITS, l * R_CORNERS:(l + 1) * R_CORNERS] = \
            protos_T / scale
    w4 = np.tile(wblk, (4, 1)).astype(BF16)           # [128, 128]
    pw = np.concatenate([planes2, w4], axis=1)        # [128, 192]

    # structural constants (shipped in bin): mask | bones4 | ident | ones32
    mask_np = (np.arange(CH)[:, None] <= np.arange(CH)[None, :]).astype(BF16)
    mask4_np = np.tile(mask_np, (1, SEQ))
    bones4_np = np.zeros((128, LR), dtype=BF16)
    for s in range(SEQ):
        for j in range(L_TABLES):
            bones4_np[32 * s + j, j * R_CORNERS:(j + 1) * R_CORNERS] = 1.0
    ones32_np = (np.arange(LR)[:, None] // R_CORNERS ==
                 (np.arange(4 * L_TABLES)[None, :] % L_TABLES)).astype(BF16)
    blob = np.concatenate([mask4_np, bones4_np,
                           np.eye(128, dtype=BF16), ones32_np], axis=1)

    in_maps = []
    for core in range(NCORES):
        base = SEQ * core
        # kt2[64u+d, 512w+t] = K^T[base + 2w+u, d, t]
        kt2 = np.empty((128, 2 * T_LEN), dtype=BF16)
        qt2 = np.empty((128, 2 * T_LEN), dtype=BF16)
        for sig in range(SEQ):
            u, w = sig % 2, sig // 2
            kt2[64 * u:64 * u + 64, T_LEN * w:T_LEN * (w + 1)] = KT[base + sig]
            qt2[64 * u:64 * u + 64, T_LEN * w:T_LEN * (w + 1)] = QT[base + sig]
        vc = np.ascontiguousarray(
            np.transpose(V4[base:base + SEQ], (2, 0, 1, 3))).astype(BF16)
        in_maps.append({
            "ain": np.concatenate([kt2, pw], axis=1),
            "bin": np.concatenate(
                [qt2, vc.reshape(CH, SEQ * NCH * D_K), blob], axis=1),
        })
    return in_maps


def kernel(Khf, Vhf, Qhf, planes_T, protos_T, _results_hook=None):
    if "nc" not in _CACHE:
        _CACHE["nc"] = _build_module()
    nc = _CACHE["nc"]
    in_maps = _host_prep(Khf, Vhf, Qhf, planes_T, protos_T)
    res = run_bass_kernel_spmd(nc, in_maps, list(range(NCORES)))
    if _results_hook is not None:
        _results_hook(res)
    out = np.empty((N_TOTAL, T_LEN, D_K), dtype=np.float32)
    for core in range(NCORES):
        out[SEQ * core:SEQ * (core + 1)] = \
            res.results[core]["out_t"].astype(np.float32)
    return np.ascontiguousarray(
        out.reshape(M_ENS, B_SZ, H_HEADS, T_LEN, D_K).transpose(0, 1, 3, 2, 4))


# revision 44
# speedup vs baseline: 1.0694x; 1.0694x over previous
"""Trainium2 Bass kernel for BatchedACE (LSH-softmax linear attention).

Math (per fused sequence n of N = M*B*H = 32):
  probs(X)[t, l, r] = softmax_r( tanh(X @ planes)/sqrt(dk) @ protos )
  A = cumsum_t(probsK)                      [T, L, R]
  S_t = cumsum_t(probsK x V outer)          [L, R, dk]
  out[t] = sum_{l,r} probsQ[t,l,r] * S_t[l,r,:] / (A[t,l,r] + 1e-6)

Key facts exploited on-chip:
  * L*R = 128 = partition dim; everything runs in [lr, t] layout.
  * chunked linear attention: per 128-chunk, out = mask(P^T Qp)^T V + Qp^T S
  * A-cumsum is a native DVE tensor_tensor_scan along the free dim.
  * |logits| <= 0.5 so softmax needs no max-subtraction.

Sharding: N=32 sequences split 4-per-core across 8 NeuronCores; no
cross-core communication.
"""
import numpy as np
import ml_dtypes
from contextlib import ExitStack

import concourse.bass as bass
import concourse.tile as tile
from concourse import bacc, mybir
from concourse.bass_utils import run_bass_kernel_spmd

BF16 = ml_dtypes.bfloat16
BF = mybir.dt.bfloat16
F32 = mybir.dt.float32
Alu = mybir.AluOpType
Act = mybir.ActivationFunctionType

M_ENS, B_SZ, T_LEN, H_HEADS, D_K = 2, 2, 512, 8, 64
K_BITS, L_TABLES, R_CORNERS = 4, 8, 16
N_TOTAL = M_ENS * B_SZ * H_HEADS          # 32
NCORES = 8
SEQ = N_TOTAL // NCORES                   # 4 sequences per core
CH = 128                                  # chunk length (partition dim)
NCH = T_LEN // CH                         # 4 chunks
LR = L_TABLES * R_CORNERS                 # 128
LK = L_TABLES * K_BITS                    # 32
EPS = 1e-6

_CACHE = {}


def _build_module(n_iters=1):
    """n_iters>1 wraps the body in a hardware For_i loop (timing builds)."""
    nc = bacc.Bacc("TRN2", target_bir_lowering=False, debug=False,
                   num_devices=NCORES)

    # per-core inputs
    kt_d = nc.dram_tensor("kt", [D_K, SEQ * T_LEN], BF, kind="ExternalInput").ap()
    qt_d = nc.dram_tensor("qt", [D_K, SEQ * T_LEN], BF, kind="ExternalInput").ap()
    v_d = nc.dram_tensor("v", [CH, SEQ * NCH * D_K], BF, kind="ExternalInput").ap()
    pw_d = nc.dram_tensor("pw", [128, LR + LK], BF, kind="ExternalInput").ap()
    out_d = nc.dram_tensor("out_t", [SEQ, D_K, T_LEN], F32, kind="ExternalOutput").ap()

    # structural constants, packed into one inline-const DMA:
    # [mask4 f32 | bf16 section bit-packed into f32 words]
    bones4_np = np.zeros((128, LR), dtype=np.float32)
    for s in range(4):
        for j in range(L_TABLES):
            bones4_np[32 * s + j, j * R_CORNERS:(j + 1) * R_CORNERS] = 1.0
    mask_np = (np.arange(CH)[:, None] <= np.arange(CH)[None, :]).astype(np.float32)
    mask4_np = np.tile(mask_np, (1, SEQ))
    ones32_np = (np.arange(LR)[:, None] // R_CORNERS ==
                 (np.arange(4 * L_TABLES)[None, :] % L_TABLES))
    bf_sec = np.concatenate([
        bones4_np.astype(BF16),                                     # 128 cols
        np.eye(128, dtype=BF16),                                    # 128 cols
        ones32_np.astype(BF16),                                     # 32 cols
    ], axis=1)                                                      # [128, 288] bf16
    bf_as_f32 = bf_sec.view(np.uint16).reshape(128, 144, 2)
    bf_words = (bf_as_f32[:, :, 0].astype(np.uint32) |
                (bf_as_f32[:, :, 1].astype(np.uint32) << 16)).view(np.float32)
    blob_np = np.concatenate([mask4_np, bf_words], axis=1)
    blob_c = nc.inline_tensor(blob_np, name="blob_c")

    with tile.TileContext(nc) as tc:
        with ExitStack() as ctx:
            cp = ctx.enter_context(tc.tile_pool(name="consts", bufs=1))
            sp = ctx.enter_context(tc.tile_pool(name="sb", bufs=1))
            lp = ctx.enter_context(tc.tile_pool(name="loop", bufs=5))
            plog = ctx.enter_context(tc.tile_pool(name="plog", bufs=1, space="PSUM"))
            pw = ctx.enter_context(tc.tile_pool(name="pw", bufs=6, space="PSUM"))
            if n_iters > 1:
                ctx.enter_context(tc.For_i(0, n_iters, 1, hint_engines=(mybir.EngineType.PE,)))

            pw_sb = cp.tile([128, LR + LK], BF)
            nc.sync.dma_start(pw_sb[:], pw_d)
            kt_sb = sp.tile([D_K, SEQ * T_LEN], BF)
            nc.sync.dma_start(kt_sb[:, 0:2 * T_LEN], kt_d[:, 0:2 * T_LEN])
            nc.sync.dma_start(kt_sb[:, 2 * T_LEN:], kt_d[:, 2 * T_LEN:])
            qt_sb = sp.tile([D_K, SEQ * T_LEN], BF)
            nc.sync.dma_start(qt_sb[:], qt_d)
            v_sb = sp.tile([CH, SEQ * NCH * D_K], BF)
            nc.sync.dma_start(v_sb[:], v_d)
            blob_sb = cp.tile([128, SEQ * CH + 144], F32)
            nc.sync.dma_start(blob_sb[:], blob_c.ap())

            w4_sb = pw_sb[:, 0:LR]
            planes_sb = pw_sb[0:D_K, LR:LR + LK]
            mask4_sb = blob_sb[:, 0:SEQ * CH]
            bf_view = blob_sb[:, SEQ * CH:SEQ * CH + 144].bitcast(BF)
            bones4_sb = bf_view[:, 0:128]
            ident_sb = bf_view[:, 128:256]
            ones32_sb = bf_view[:, 256:288]

            def S(s):
                return slice(T_LEN * s, T_LEN * (s + 1))

            # ---- probs pipelines: full K chain first, then Q ----
            xt = {"k": kt_sb, "q": qt_sb}
            dst = {}
            dst["k"] = sp.tile([128, SEQ * T_LEN], BF, tag="ptk", name="ptk")
            dst["q"] = sp.tile([128, SEQ * T_LEN], BF, tag="qeq", name="qeq")

            def probs_chain(x):
                proj_ps = pw.tile([128, T_LEN], F32, tag="w", name=f"proj{x}")
                for s in range(SEQ):
                    nc.tensor.matmul(proj_ps[32 * s:32 * s + 32, :],
                                     planes_sb, xt[x][:, S(s)],
                                     start=True, stop=True,
                                     tile_position=(0, 32 * s))
                tanh_sb = lp.tile([128, T_LEN], BF, tag=f"tanh{x}", name=f"tanh{x}")
                nc.scalar.activation(tanh_sb[:], proj_ps[:], Act.Tanh)

                e_sb = sp.tile([128, SEQ * T_LEN], BF, tag=f"e{x}", name=f"e{x}")
                sums_ps = pw.tile([128, T_LEN], F32, tag="w", name=f"sums{x}")
                for s in range(SEQ):
                    logit_ps = pw.tile([128, T_LEN], F32, tag="w",
                                       name=f"log{x}{s}")
                    nc.tensor.matmul(logit_ps[:],
                                     w4_sb[32 * s:32 * s + 32, :],
                                     tanh_sb[32 * s:32 * s + 32, :],
                                     start=True, stop=True,
                                     tile_position=(32 * s, 0))
                    nc.scalar.activation(e_sb[:, S(s)], logit_ps[:], Act.Exp)
                    nc.tensor.matmul(sums_ps[32 * s:32 * s + 32, :],
                                     ones32_sb, e_sb[:, S(s)],
                                     start=True, stop=True,
                                     tile_position=(0, 32 * s))
                # reciprocal of softmax sums -> bf16 -> broadcast over the
                # 16 corners via a block-ones matmul, then normalize on DVE
                recip_f = lp.tile([128, T_LEN], F32, tag=f"recipf{x}",
                                  name=f"recipf{x}")
                recip_b = lp.tile([128, T_LEN], BF, tag=f"recip{x}",
                                  name=f"recip{x}")
                nc.vector.reciprocal_approx_fast(recip_f[:], sums_ps[:])
                nc.scalar.copy(recip_b[:], recip_f[:])
                for h in range(2):
                    b = plog.tile([128, 2 * T_LEN], F32, tag="log",
                                  name=f"bc{x}{h}")
                    for i in range(2):
                        s = 2 * h + i
                        nc.tensor.matmul(b[:, T_LEN * i:T_LEN * (i + 1)],
                                         bones4_sb[32 * s:32 * s + 8, :],
                                         recip_b[32 * s:32 * s + 8, :],
                                         start=True, stop=True,
                                         tile_position=(32 * s, 0))
                    cols = slice(T_LEN * 2 * h, T_LEN * 2 * (h + 1))
                    nc.vector.tensor_mul(dst[x][:, cols], e_sb[:, cols], b[:])
            probs_chain("k")
            pt_sb = dst["k"]
            probs_chain("q")
            qe_sb = dst["q"]

            # ---- chunked attention ----
            def tsl(s, c):
                return slice(T_LEN * s + CH * c, T_LEN * s + CH * (c + 1))

            def vsl(s, c):
                return slice(D_K * (s * NCH + c), D_K * (s * NCH + c + 1))

            # A = cumsum(P) + eps, then Qp (per seq, all on DVE)
            a_sb = sp.tile([128, SEQ * T_LEN], F32)
            ra_sb = sp.tile([128, SEQ * T_LEN], F32)
            qp_sb = sp.tile([128, SEQ * T_LEN], BF)
            for s in range(SEQ):
                nc.vector.tensor_tensor_scan(a_sb[:, S(s)], pt_sb[:, S(s)],
                                             pt_sb[:, S(s)], EPS,
                                             Alu.add, Alu.bypass)
                nc.vector.reciprocal_approx_fast(ra_sb[:, S(s)], a_sb[:, S(s)])
                nc.vector.tensor_mul(qp_sb[:, S(s)], qe_sb[:, S(s)],
                                     ra_sb[:, S(s)])

            # intra-chunk quadratic term + P transposes
            gm_sb, pn_sb = {}, {}
            for c in range(NCH):
                gt_ps = pw.tile([CH, SEQ * CH], F32, tag="w")
                for s in range(SEQ):
                    nc.tensor.matmul(gt_ps[:, CH * s:CH * (s + 1)],
                                     pt_sb[:, tsl(s, c)], qp_sb[:, tsl(s, c)],
                                     start=True, stop=True)
                gm_sb[c] = lp.tile([CH, SEQ * CH], BF, tag="gm", name=f"gm{c}")
                nc.vector.tensor_mul(gm_sb[c][:], gt_ps[:], mask4_sb[:])

                if c < NCH - 1:
                    tr_ps = pw.tile([CH, SEQ * CH], BF, tag="w")
                    for s in range(SEQ):
                        nc.tensor.transpose(tr_ps[:, CH * s:CH * (s + 1)],
                                            pt_sb[:, tsl(s, c)], ident_sb[:])
                    pn_sb[c] = lp.tile([CH, SEQ * CH], BF, tag="pn", name=f"pn{c}")
                    nc.scalar.copy(pn_sb[c][:], tr_ps[:])

            s_tiles = []
            s_prev = None
            for c in range(NCH - 1):
                ds_ps = pw.tile([LR, SEQ * D_K], F32, tag="w", name=f"dsp{c}")
                for s in range(SEQ):
                    nc.tensor.matmul(ds_ps[:, D_K * s:D_K * (s + 1)],
                                     pn_sb[c][:, CH * s:CH * (s + 1)],
                                     v_sb[:, vsl(s, c)],
                                     start=True, stop=True)
                s_new = sp.tile([LR, SEQ * D_K], BF, tag=f"state{c}",
                                name=f"state{c}")
                if s_prev is None:
                    nc.scalar.copy(s_new[:], ds_ps[:])
                else:
                    nc.vector.tensor_add(s_new[:], ds_ps[:], s_prev[:])
                s_tiles.append(s_new)
                s_prev = s_new

            for c in range(NCH):
                out_ps = pw.tile([D_K, SEQ * CH], F32, tag="w")
                for s in range(SEQ):
                    nc.tensor.matmul(out_ps[:, CH * s:CH * (s + 1)],
                                     v_sb[:, vsl(s, c)],
                                     gm_sb[c][:, CH * s:CH * (s + 1)],
                                     start=True, stop=(c == 0))
                    if c > 0:
                        nc.tensor.matmul(out_ps[:, CH * s:CH * (s + 1)],
                                         s_tiles[c - 1][:, D_K * s:D_K * (s + 1)],
                                         qp_sb[:, tsl(s, c)],
                                         start=False, stop=True)
                out_sb = lp.tile([D_K, SEQ * CH], F32, tag="osb")
                nc.scalar.copy(out_sb[:], out_ps[:])
                nc.sync.dma_start(
                    out_d[:, :, CH * c:CH * (c + 1)].rearrange("s d t -> d s t"),
                    out_sb[:].rearrange("d (s t) -> d s t", s=SEQ))

    nc.compile()
    return nc


def _host_prep(Khf, Vhf, Qhf, planes_T, protos_T):
    """Fold + transpose + quantize inputs; build per-core in_maps."""
    Khf = np.asarray(Khf, dtype=np.float32)
    Vhf = np.asarray(Vhf, dtype=np.float32)
    Qhf = np.asarray(Qhf, dtype=np.float32)
    planes_T = np.asarray(planes_T, dtype=np.float32)
    protos_T = np.asarray(protos_T, dtype=np.float32)
    scale = np.sqrt(np.float32(D_K))

    def fold(x):
        return np.transpose(x, (0, 1, 3, 2, 4)).reshape(N_TOTAL, T_LEN, D_K)

    K2, Q2, V2 = fold(Khf), fold(Qhf), fold(Vhf)
    KT = np.ascontiguousarray(np.transpose(K2, (0, 2, 1))).astype(BF16)  # [N, dk, T]
    QT = np.ascontiguousarray(np.transpose(Q2, (0, 2, 1))).astype(BF16)
    V4 = V2.reshape(N_TOTAL, NCH, CH, D_K)

    w4 = np.zeros((128, LR), dtype=np.float32)
    wblk = np.zeros((LK, LR), dtype=np.float32)
    for l in range(L_TABLES):
        wblk[l * K_BITS:(l + 1) * K_BITS, l * R_CORNERS:(l + 1) * R_CORNERS] = \
            protos_T / scale
    for s in range(4):
        w4[32 * s:32 * s + 32, :] = wblk
    pw = np.zeros((128, LR + LK), dtype=BF16)
    pw[:, 0:LR] = w4.astype(BF16)
    pw[0:D_K, LR:LR + LK] = planes_T.astype(BF16)

    in_maps = []
    for core in range(NCORES):
        ns = slice(SEQ * core, SEQ * (core + 1))
        ktc = np.ascontiguousarray(KT[ns]).reshape(SEQ, D_K, T_LEN)
        qtc = np.ascontiguousarray(QT[ns]).reshape(SEQ, D_K, T_LEN)
        vc = np.ascontiguousarray(
            np.transpose(V4[ns], (2, 0, 1, 3))).astype(BF16)  # [128, seq, ch, dk]
        in_maps.append({
            "kt": np.ascontiguousarray(np.transpose(ktc, (1, 0, 2))).reshape(D_K, SEQ * T_LEN),
            "qt": np.ascontiguousarray(np.transpose(qtc, (1, 0, 2))).reshape(D_K, SEQ * T_LEN),
            "v": vc.reshape(CH, SEQ * NCH * D_K),
            "pw": pw,
        })
    return in_maps


def kernel(Khf, Vhf, Qhf, planes_T, protos_T, _results_hook=None):
    if "nc" not in _CACHE:
        _CACHE["nc"] = _build_module()
    nc = _CACHE["nc"]
    in_maps = _host_prep(Khf, Vhf, Qhf, planes_T, protos_T)
    res = run_bass_kernel_spmd(nc, in_maps, list(range(NCORES)))
    if _results_hook is not None:
        _results_hook(res)
    out = np.empty((N_TOTAL, T_LEN, D_K), dtype=np.float32)
    for core in range(NCORES):
        out_t = res.results[core]["out_t"]          # [SEQ, dk, T]
        out[SEQ * core:SEQ * (core + 1)] = np.transpose(out_t, (0, 2, 1))
    return np.ascontiguousarray(
        out.reshape(M_ENS, B_SZ, H_HEADS, T_LEN, D_K).transpose(0, 1, 3, 2, 4))



# revision 47
# speedup vs baseline: 1.1154x; 1.0430x over previous
"""Trainium2 Bass kernel for BatchedACE (LSH-softmax linear attention).

Math (per fused sequence n of N = M*B*H = 32):
  probs(X)[t, l, r] = softmax_r( tanh(X @ planes)/sqrt(dk) @ protos )
  A = cumsum_t(probsK)                      [T, L, R]
  S_t = cumsum_t(probsK x V outer)          [L, R, dk]
  out[t] = sum_{l,r} probsQ[t,l,r] * S_t[l,r,:] / (A[t,l,r] + 1e-6)

Key facts exploited on-chip:
  * L*R = 128 = partition dim; everything runs in [lr, t] layout.
  * chunked linear attention: per 128-chunk, out = mask(P^T Qp)^T V + Qp^T S
  * A-cumsum is a native DVE tensor_tensor_scan along the free dim.
  * |logits| <= 0.5 so softmax needs no max-subtraction.

Sharding: N=32 sequences split 4-per-core across 8 NeuronCores; no
cross-core communication.
"""
import numpy as np
import ml_dtypes
from contextlib import ExitStack

import concourse.bass as bass
import concourse.tile as tile
from concourse import bacc, mybir
from concourse.bass_utils import run_bass_kernel_spmd

BF16 = ml_dtypes.bfloat16
BF = mybir.dt.bfloat16
F32 = mybir.dt.float32
Alu = mybir.AluOpType
Act = mybir.ActivationFunctionType

M_ENS, B_SZ, T_LEN, H_HEADS, D_K = 2, 2, 512, 8, 64
K_BITS, L_TABLES, R_CORNERS = 4, 8, 16
N_TOTAL = M_ENS * B_SZ * H_HEADS          # 32
NCORES = 8
SEQ = N_TOTAL // NCORES                   # 4 sequences per core
CH = 128                                  # chunk length (partition dim)
NCH = T_LEN // CH                         # 4 chunks
LR = L_TABLES * R_CORNERS                 # 128
LK = L_TABLES * K_BITS                    # 32
EPS = 1e-6

_CACHE = {}


def _build_module(n_iters=1):
    """n_iters>1 wraps the body in a hardware For_i loop (timing builds)."""
    nc = bacc.Bacc("TRN2", target_bir_lowering=False, debug=False,
                   num_devices=NCORES)

    # per-core inputs
    kt_d = nc.dram_tensor("kt", [D_K, SEQ * T_LEN], BF, kind="ExternalInput").ap()
    qt_d = nc.dram_tensor("qt", [D_K, SEQ * T_LEN], BF, kind="ExternalInput").ap()
    v_d = nc.dram_tensor("v", [CH, SEQ * NCH * D_K], BF, kind="ExternalInput").ap()
    pw_d = nc.dram_tensor("pw", [128, LR + LK], BF, kind="ExternalInput").ap()
    out_d = nc.dram_tensor("out_t", [SEQ, D_K, T_LEN], BF, kind="ExternalOutput").ap()

    # structural constants, packed into one inline-const DMA:
    # [mask4 f32 | bf16 section bit-packed into f32 words]
    bones4_np = np.zeros((128, LR), dtype=np.float32)
    for s in range(4):
        for j in range(L_TABLES):
            bones4_np[32 * s + j, j * R_CORNERS:(j + 1) * R_CORNERS] = 1.0
    mask_np = (np.arange(CH)[:, None] <= np.arange(CH)[None, :]).astype(np.float32)
    mask4_np = np.tile(mask_np, (1, SEQ))
    ones32_np = (np.arange(LR)[:, None] // R_CORNERS ==
                 (np.arange(4 * L_TABLES)[None, :] % L_TABLES))
    bf_sec = np.concatenate([
        bones4_np.astype(BF16),                                     # 128 cols
        np.eye(128, dtype=BF16),                                    # 128 cols
        ones32_np.astype(BF16),                                     # 32 cols
    ], axis=1)                                                      # [128, 288] bf16
    bf_as_f32 = bf_sec.view(np.uint16).reshape(128, 144, 2)
    bf_words = (bf_as_f32[:, :, 0].astype(np.uint32) |
                (bf_as_f32[:, :, 1].astype(np.uint32) << 16)).view(np.float32)
    blob_np = np.concatenate([mask4_np, bf_words], axis=1)
    blob_c = nc.inline_tensor(blob_np, name="blob_c")

    with tile.TileContext(nc) as tc:
        with ExitStack() as ctx:
            cp = ctx.enter_context(tc.tile_pool(name="consts", bufs=1))
            sp = ctx.enter_context(tc.tile_pool(name="sb", bufs=1))
            lp = ctx.enter_context(tc.tile_pool(name="loop", bufs=5))
            plog = ctx.enter_context(tc.tile_pool(name="plog", bufs=1, space="PSUM"))
            pw = ctx.enter_context(tc.tile_pool(name="pw", bufs=6, space="PSUM"))
            if n_iters > 1:
                ctx.enter_context(tc.For_i(0, n_iters, 1, hint_engines=(mybir.EngineType.PE,)))

            pw_sb = cp.tile([128, LR + LK], BF)
            nc.sync.dma_start(pw_sb[:], pw_d)
            kt_sb = sp.tile([D_K, SEQ * T_LEN], BF)
            nc.sync.dma_start(kt_sb[:, 0:2 * T_LEN], kt_d[:, 0:2 * T_LEN])
            nc.sync.dma_start(kt_sb[:, 2 * T_LEN:], kt_d[:, 2 * T_LEN:])
            qt_sb = sp.tile([D_K, SEQ * T_LEN], BF)
            nc.sync.dma_start(qt_sb[:], qt_d)
            v_sb = sp.tile([CH, SEQ * NCH * D_K], BF)
            nc.sync.dma_start(v_sb[:], v_d)
            blob_sb = cp.tile([128, SEQ * CH + 144], F32)
            nc.sync.dma_start(blob_sb[:], blob_c.ap())

            w4_sb = pw_sb[:, 0:LR]
            planes_sb = pw_sb[0:D_K, LR:LR + LK]
            mask4_sb = blob_sb[:, 0:SEQ * CH]
            bf_view = blob_sb[:, SEQ * CH:SEQ * CH + 144].bitcast(BF)
            bones4_sb = bf_view[:, 0:128]
            ident_sb = bf_view[:, 128:256]
            ones32_sb = bf_view[:, 256:288]

            def S(s):
                return slice(T_LEN * s, T_LEN * (s + 1))

            # ---- probs pipelines: full K chain first, then Q ----
            xt = {"k": kt_sb, "q": qt_sb}
            dst = {}
            dst["k"] = sp.tile([128, SEQ * T_LEN], BF, tag="ptk", name="ptk")
            dst["q"] = sp.tile([128, SEQ * T_LEN], BF, tag="qeq", name="qeq")

            tanh_t = {}

            def probs_proj(x):
                proj_ps = pw.tile([128, T_LEN], F32, tag="w", name=f"proj{x}")
                for s in range(SEQ):
                    nc.tensor.matmul(proj_ps[32 * s:32 * s + 32, :],
                                     planes_sb, xt[x][:, S(s)],
                                     start=True, stop=True,
                                     tile_position=(0, 32 * s))
                return proj_ps

            def probs_tanh(x, proj_ps):
                tanh_t[x] = lp.tile([128, T_LEN], BF, tag=f"tanh{x}",
                                    name=f"tanh{x}")
                nc.scalar.activation(tanh_t[x][:], proj_ps[:], Act.Tanh)

            def probs_chain(x):
                tanh_sb = tanh_t[x]
                e_sb = sp.tile([128, SEQ * T_LEN], BF, tag=f"e{x}", name=f"e{x}")
                sums_ps = pw.tile([128, T_LEN], F32, tag="w", name=f"sums{x}")
                for s in range(SEQ):
                    logit_ps = pw.tile([128, T_LEN], F32, tag="w",
                                       name=f"log{x}{s}")
                    nc.tensor.matmul(logit_ps[:],
                                     w4_sb[32 * s:32 * s + 32, :],
                                     tanh_sb[32 * s:32 * s + 32, :],
                                     start=True, stop=True,
                                     tile_position=(32 * s, 0))
                    nc.scalar.activation(e_sb[:, S(s)], logit_ps[:], Act.Exp)
                    nc.tensor.matmul(sums_ps[32 * s:32 * s + 32, :],
                                     ones32_sb, e_sb[:, S(s)],
                                     start=True, stop=True,
                                     tile_position=(0, 32 * s))
                # reciprocal of softmax sums -> bf16 -> broadcast over the
                # 16 corners via a block-ones matmul, then normalize on DVE
                recip_f = lp.tile([128, T_LEN], F32, tag=f"recipf{x}",
                                  name=f"recipf{x}")
                recip_b = lp.tile([128, T_LEN], BF, tag=f"recip{x}",
                                  name=f"recip{x}")
                nc.vector.reciprocal_approx_fast(recip_f[:], sums_ps[:])
                nc.scalar.copy(recip_b[:], recip_f[:])
                for h in range(2):
                    b = plog.tile([128, 2 * T_LEN], F32, tag="log",
                                  name=f"bc{x}{h}")
                    for i in range(2):
                        s = 2 * h + i
                        nc.tensor.matmul(b[:, T_LEN * i:T_LEN * (i + 1)],
                                         bones4_sb[32 * s:32 * s + 8, :],
                                         recip_b[32 * s:32 * s + 8, :],
                                         start=True, stop=True,
                                         tile_position=(32 * s, 0))
                    cols = slice(T_LEN * 2 * h, T_LEN * 2 * (h + 1))
                    nc.vector.tensor_mul(dst[x][:, cols], e_sb[:, cols], b[:])
            pjk = probs_proj("k")
            probs_tanh("k", pjk)
            probs_chain("k")
            pt_sb = dst["k"]
            pjq = probs_proj("q")
            probs_tanh("q", pjq)
            probs_chain("q")
            qe_sb = dst["q"]

            # ---- chunked attention ----
            def tsl(s, c):
                return slice(T_LEN * s + CH * c, T_LEN * s + CH * (c + 1))

            def vsl(s, c):
                return slice(D_K * (s * NCH + c), D_K * (s * NCH + c + 1))

            # A = cumsum(P) + eps, then Qp (per seq, all on DVE)
            a_sb = sp.tile([128, SEQ * T_LEN], F32)
            ra_sb = sp.tile([128, SEQ * T_LEN], F32)
            qp_sb = sp.tile([128, SEQ * T_LEN], BF)
            for s in range(SEQ):
                nc.vector.tensor_tensor_scan(a_sb[:, S(s)], pt_sb[:, S(s)],
                                             pt_sb[:, S(s)], EPS,
                                             Alu.add, Alu.bypass)
                nc.vector.reciprocal_approx_fast(ra_sb[:, S(s)], a_sb[:, S(s)])
                nc.vector.tensor_mul(qp_sb[:, S(s)], qe_sb[:, S(s)],
                                     ra_sb[:, S(s)])

            # intra-chunk quadratic term + P transposes
            gm_sb, pn_sb = {}, {}
            for c in range(NCH):
                gt_ps = pw.tile([CH, SEQ * CH], F32, tag="w")
                for s in range(SEQ):
                    nc.tensor.matmul(gt_ps[:, CH * s:CH * (s + 1)],
                                     pt_sb[:, tsl(s, c)], qp_sb[:, tsl(s, c)],
                                     start=True, stop=True)
                gm_sb[c] = lp.tile([CH, SEQ * CH], BF, tag="gm", name=f"gm{c}")
                nc.vector.tensor_mul(gm_sb[c][:], gt_ps[:], mask4_sb[:])

                if c < NCH - 1:
                    tr_ps = pw.tile([CH, SEQ * CH], BF, tag="w")
                    for s in range(SEQ):
                        nc.tensor.transpose(tr_ps[:, CH * s:CH * (s + 1)],
                                            pt_sb[:, tsl(s, c)], ident_sb[:])
                    pn_sb[c] = lp.tile([CH, SEQ * CH], BF, tag="pn", name=f"pn{c}")
                    nc.scalar.copy(pn_sb[c][:], tr_ps[:])

            s_tiles = []
            s_prev = None
            for c in range(NCH - 1):
                ds_ps = pw.tile([LR, SEQ * D_K], F32, tag="w", name=f"dsp{c}")
                for s in range(SEQ):
                    nc.tensor.matmul(ds_ps[:, D_K * s:D_K * (s + 1)],
                                     pn_sb[c][:, CH * s:CH * (s + 1)],
                                     v_sb[:, vsl(s, c)],
                                     start=True, stop=True)
                s_new = sp.tile([LR, SEQ * D_K], BF, tag=f"state{c}",
                                name=f"state{c}")
                if s_prev is None:
                    nc.scalar.copy(s_new[:], ds_ps[:])
                else:
                    nc.vector.tensor_add(s_new[:], ds_ps[:], s_prev[:])
                s_tiles.append(s_new)
                s_prev = s_new

            for c in range(NCH):
                out_ps = pw.tile([D_K, SEQ * CH], F32, tag="w")
                for s in range(SEQ):
                    nc.tensor.matmul(out_ps[:, CH * s:CH * (s + 1)],
                                     v_sb[:, vsl(s, c)],
                                     gm_sb[c][:, CH * s:CH * (s + 1)],
                                     start=True, stop=(c == 0))
                    if c > 0:
                        nc.tensor.matmul(out_ps[:, CH * s:CH * (s + 1)],
                                         s_tiles[c - 1][:, D_K * s:D_K * (s + 1)],
                                         qp_sb[:, tsl(s, c)],
                                         start=False, stop=True)
                out_sb = lp.tile([D_K, SEQ * CH], BF, tag="osb")
                nc.scalar.copy(out_sb[:], out_ps[:])
                nc.sync.dma_start(
                    out_d[:, :, CH * c:CH * (c + 1)].rearrange("s d t -> d s t"),
                    out_sb[:].rearrange("d (s t) -> d s t", s=SEQ))

    nc.compile()
    return nc


def _host_prep(Khf, Vhf, Qhf, planes_T, protos_T):
    """Fold + transpose + quantize inputs; build per-core in_maps."""
    Khf = np.asarray(Khf, dtype=np.float32)
    Vhf = np.asarray(Vhf, dtype=np.float32)
    Qhf = np.asarray(Qhf, dtype=np.float32)
    planes_T = np.asarray(planes_T, dtype=np.float32)
    protos_T = np.asarray(protos_T, dtype=np.float32)
    scale = np.sqrt(np.float32(D_K))

    def fold(x):
        return np.transpose(x, (0, 1, 3, 2, 4)).reshape(N_TOTAL, T_LEN, D_K)

    K2, Q2, V2 = fold(Khf), fold(Qhf), fold(Vhf)
    KT = np.ascontiguousarray(np.transpose(K2, (0, 2, 1))).astype(BF16)  # [N, dk, T]
    QT = np.ascontiguousarray(np.transpose(Q2, (0, 2, 1))).astype(BF16)
    V4 = V2.reshape(N_TOTAL, NCH, CH, D_K)

    w4 = np.zeros((128, LR), dtype=np.float32)
    wblk = np.zeros((LK, LR), dtype=np.float32)
    for l in range(L_TABLES):
        wblk[l * K_BITS:(l + 1) * K_BITS, l * R_CORNERS:(l + 1) * R_CORNERS] = \
            protos_T / scale
    for s in range(4):
        w4[32 * s:32 * s + 32, :] = wblk
    pw = np.zeros((128, LR + LK), dtype=BF16)
    pw[:, 0:LR] = w4.astype(BF16)
    pw[0:D_K, LR:LR + LK] = planes_T.astype(BF16)

    in_maps = []
    for core in range(NCORES):
        ns = slice(SEQ * core, SEQ * (core + 1))
        ktc = np.ascontiguousarray(KT[ns]).reshape(SEQ, D_K, T_LEN)
        qtc = np.ascontiguousarray(QT[ns]).reshape(SEQ, D_K, T_LEN)
        vc = np.ascontiguousarray(
            np.transpose(V4[ns], (2, 0, 1, 3))).astype(BF16)  # [128, seq, ch, dk]
        in_maps.append({
            "kt": np.ascontiguousarray(np.transpose(ktc, (1, 0, 2))).reshape(D_K, SEQ * T_LEN),
            "qt": np.ascontiguousarray(np.transpose(qtc, (1, 0, 2))).reshape(D_K, SEQ * T_LEN),
            "v": vc.reshape(CH, SEQ * NCH * D_K),
            "pw": pw,
        })
    return in_maps


def kernel(Khf, Vhf, Qhf, planes_T, protos_T, _results_hook=None):
    if "nc" not in _CACHE:
        _CACHE["nc"] = _build_module()
    nc = _CACHE["nc"]
    in_maps = _host_prep(Khf, Vhf, Qhf, planes_T, protos_T)
    res = run_bass_kernel_spmd(nc, in_maps, list(range(NCORES)))
    if _results_hook is not None:
        _results_hook(res)
    out = np.empty((N_TOTAL, T_LEN, D_K), dtype=np.float32)
    for core in range(NCORES):
        out_t = res.results[core]["out_t"].astype(np.float32)  # [SEQ, dk, T]
        out[SEQ * core:SEQ * (core + 1)] = np.transpose(out_t, (0, 2, 1))
    return np.ascontiguousarray(
        out.reshape(M_ENS, B_SZ, H_HEADS, T_LEN, D_K).transpose(0, 1, 3, 2, 4))



# revision 48
# speedup vs baseline: 1.1229x; 1.0068x over previous
"""Trainium2 Bass kernel for BatchedACE (LSH-softmax linear attention).

Math (per fused sequence n of N = M*B*H = 32):
  probs(X)[t, l, r] = softmax_r( tanh(X @ planes)/sqrt(dk) @ protos )
  A = cumsum_t(probsK)                      [T, L, R]
  S_t = cumsum_t(probsK x V outer)          [L, R, dk]
  out[t] = sum_{l,r} probsQ[t,l,r] * S_t[l,r,:] / (A[t,l,r] + 1e-6)

Key facts exploited on-chip:
  * L*R = 128 = partition dim; everything runs in [lr, t] layout.
  * chunked linear attention: per 128-chunk, out = mask(P^T Qp)^T V + Qp^T S
  * A-cumsum is a native DVE tensor_tensor_scan along the free dim.
  * |logits| <= 0.5 so softmax needs no max-subtraction.

Sharding: N=32 sequences split 4-per-core across 8 NeuronCores; no
cross-core communication.
"""
import numpy as np
import ml_dtypes
from contextlib import ExitStack

import concourse.bass as bass
import concourse.tile as tile
from concourse import bacc, mybir
from concourse.bass_utils import run_bass_kernel_spmd

BF16 = ml_dtypes.bfloat16
BF = mybir.dt.bfloat16
F32 = mybir.dt.float32
Alu = mybir.AluOpType
Act = mybir.ActivationFunctionType

M_ENS, B_SZ, T_LEN, H_HEADS, D_K = 2, 2, 512, 8, 64
K_BITS, L_TABLES, R_CORNERS = 4, 8, 16
N_TOTAL = M_ENS * B_SZ * H_HEADS          # 32
NCORES = 8
SEQ = N_TOTAL // NCORES                   # 4 sequences per core
CH = 128                                  # chunk length (partition dim)
NCH = T_LEN // CH                         # 4 chunks
LR = L_TABLES * R_CORNERS                 # 128
LK = L_TABLES * K_BITS                    # 32
EPS = 1e-6

_CACHE = {}


def _build_module(n_iters=1):
    """n_iters>1 wraps the body in a hardware For_i loop (timing builds)."""
    nc = bacc.Bacc("TRN2", target_bir_lowering=False, debug=False,
                   num_devices=NCORES)

    # per-core inputs
    kt_d = nc.dram_tensor("kt", [D_K, SEQ * T_LEN], BF, kind="ExternalInput").ap()
    qt_d = nc.dram_tensor("qt", [D_K, SEQ * T_LEN], BF, kind="ExternalInput").ap()
    v_d = nc.dram_tensor("v", [CH, SEQ * NCH * D_K], BF, kind="ExternalInput").ap()
    pw_d = nc.dram_tensor("pw", [128, LR + LK], BF, kind="ExternalInput").ap()
    out_d = nc.dram_tensor("out_t", [SEQ, D_K, T_LEN], BF, kind="ExternalOutput").ap()

    # structural constants, all bf16, bit-packed into one f32 inline-const
    # DMA: [mask4 (512) | bones4 (128) | ident (128) | ones32 (32)]
    bones4_np = np.zeros((128, LR), dtype=np.float32)
    for s in range(4):
        for j in range(L_TABLES):
            bones4_np[32 * s + j, j * R_CORNERS:(j + 1) * R_CORNERS] = 1.0
    mask_np = (np.arange(CH)[:, None] <= np.arange(CH)[None, :]).astype(BF16)
    mask4_np = np.tile(mask_np, (1, SEQ))
    ones32_np = (np.arange(LR)[:, None] // R_CORNERS ==
                 (np.arange(4 * L_TABLES)[None, :] % L_TABLES))
    bf_sec = np.concatenate([
        mask4_np,                                                   # 512 cols
        bones4_np.astype(BF16),                                     # 128 cols
        np.eye(128, dtype=BF16),                                    # 128 cols
        ones32_np.astype(BF16),                                     # 32 cols
    ], axis=1)                                                      # [128, 800] bf16
    bf_as_f32 = bf_sec.view(np.uint16).reshape(128, 400, 2)
    blob_np = (bf_as_f32[:, :, 0].astype(np.uint32) |
               (bf_as_f32[:, :, 1].astype(np.uint32) << 16)).view(np.float32)
    blob_c = nc.inline_tensor(blob_np, name="blob_c")

    with tile.TileContext(nc) as tc:
        with ExitStack() as ctx:
            cp = ctx.enter_context(tc.tile_pool(name="consts", bufs=1))
            sp = ctx.enter_context(tc.tile_pool(name="sb", bufs=1))
            lp = ctx.enter_context(tc.tile_pool(name="loop", bufs=5))
            plog = ctx.enter_context(tc.tile_pool(name="plog", bufs=1, space="PSUM"))
            pw = ctx.enter_context(tc.tile_pool(name="pw", bufs=6, space="PSUM"))
            if n_iters > 1:
                ctx.enter_context(tc.For_i(0, n_iters, 1, hint_engines=(mybir.EngineType.PE,)))

            pw_sb = cp.tile([128, LR + LK], BF)
            nc.sync.dma_start(pw_sb[:], pw_d)
            kt_sb = sp.tile([D_K, SEQ * T_LEN], BF)
            nc.sync.dma_start(kt_sb[:, 0:2 * T_LEN], kt_d[:, 0:2 * T_LEN])
            nc.sync.dma_start(kt_sb[:, 2 * T_LEN:], kt_d[:, 2 * T_LEN:])
            qt_sb = sp.tile([D_K, SEQ * T_LEN], BF)
            nc.sync.dma_start(qt_sb[:], qt_d)
            v_sb = sp.tile([CH, SEQ * NCH * D_K], BF)
            nc.sync.dma_start(v_sb[:], v_d)
            blob_sb = cp.tile([128, 400], F32)
            nc.sync.dma_start(blob_sb[:], blob_c.ap())

            w4_sb = pw_sb[:, 0:LR]
            planes_sb = pw_sb[0:D_K, LR:LR + LK]
            bf_view = blob_sb[:].bitcast(BF)
            mask4_sb = bf_view[:, 0:512]
            bones4_sb = bf_view[:, 512:640]
            ident_sb = bf_view[:, 640:768]
            ones32_sb = bf_view[:, 768:800]

            def S(s):
                return slice(T_LEN * s, T_LEN * (s + 1))

            # ---- probs pipelines: full K chain first, then Q ----
            xt = {"k": kt_sb, "q": qt_sb}
            dst = {}
            dst["k"] = sp.tile([128, SEQ * T_LEN], BF, tag="ptk", name="ptk")
            dst["q"] = sp.tile([128, SEQ * T_LEN], BF, tag="qeq", name="qeq")

            tanh_t = {}

            def probs_proj(x):
                proj_ps = pw.tile([128, T_LEN], F32, tag="w", name=f"proj{x}")
                for s in range(SEQ):
                    nc.tensor.matmul(proj_ps[32 * s:32 * s + 32, :],
                                     planes_sb, xt[x][:, S(s)],
                                     start=True, stop=True,
                                     tile_position=(0, 32 * s))
                return proj_ps

            def probs_tanh(x, proj_ps):
                tanh_t[x] = lp.tile([128, T_LEN], BF, tag=f"tanh{x}",
                                    name=f"tanh{x}")
                nc.scalar.activation(tanh_t[x][:], proj_ps[:], Act.Tanh)

            def probs_chain(x):
                tanh_sb = tanh_t[x]
                e_sb = sp.tile([128, SEQ * T_LEN], BF, tag=f"e{x}", name=f"e{x}")
                sums_ps = pw.tile([128, T_LEN], F32, tag="w", name=f"sums{x}")
                for s in range(SEQ):
                    logit_ps = pw.tile([128, T_LEN], F32, tag="w",
                                       name=f"log{x}{s}")
                    nc.tensor.matmul(logit_ps[:],
                                     w4_sb[32 * s:32 * s + 32, :],
                                     tanh_sb[32 * s:32 * s + 32, :],
                                     start=True, stop=True,
                                     tile_position=(32 * s, 0))
                    nc.scalar.activation(e_sb[:, S(s)], logit_ps[:], Act.Exp)
                    nc.tensor.matmul(sums_ps[32 * s:32 * s + 32, :],
                                     ones32_sb, e_sb[:, S(s)],
                                     start=True, stop=True,
                                     tile_position=(0, 32 * s))
                # reciprocal of softmax sums -> bf16 -> broadcast over the
                # 16 corners via a block-ones matmul, then normalize on DVE
                recip_f = lp.tile([128, T_LEN], F32, tag=f"recipf{x}",
                                  name=f"recipf{x}")
                recip_b = lp.tile([128, T_LEN], BF, tag=f"recip{x}",
                                  name=f"recip{x}")
                nc.vector.reciprocal_approx_fast(recip_f[:], sums_ps[:])
                nc.scalar.copy(recip_b[:], recip_f[:])
                for h in range(2):
                    b = plog.tile([128, 2 * T_LEN], F32, tag="log",
                                  name=f"bc{x}{h}")
                    for i in range(2):
                        s = 2 * h + i
                        nc.tensor.matmul(b[:, T_LEN * i:T_LEN * (i + 1)],
                                         bones4_sb[32 * s:32 * s + 8, :],
                                         recip_b[32 * s:32 * s + 8, :],
                                         start=True, stop=True,
                                         tile_position=(32 * s, 0))
                    cols = slice(T_LEN * 2 * h, T_LEN * 2 * (h + 1))
                    nc.vector.tensor_mul(dst[x][:, cols], e_sb[:, cols], b[:])
            pjk = probs_proj("k")
            probs_tanh("k", pjk)
            probs_chain("k")
            pt_sb = dst["k"]
            pjq = probs_proj("q")
            probs_tanh("q", pjq)
            probs_chain("q")
            qe_sb = dst["q"]

            # ---- chunked attention ----
            def tsl(s, c):
                return slice(T_LEN * s + CH * c, T_LEN * s + CH * (c + 1))

            def vsl(s, c):
                return slice(D_K * (s * NCH + c), D_K * (s * NCH + c + 1))

            # A = cumsum(P) + eps, then Qp (per seq, all on DVE)
            a_sb = sp.tile([128, SEQ * T_LEN], F32)
            ra_sb = sp.tile([128, SEQ * T_LEN], F32)
            qp_sb = sp.tile([128, SEQ * T_LEN], BF)
            for s in range(SEQ):
                nc.vector.tensor_tensor_scan(a_sb[:, S(s)], pt_sb[:, S(s)],
                                             pt_sb[:, S(s)], EPS,
                                             Alu.add, Alu.bypass)
                nc.vector.reciprocal_approx_fast(ra_sb[:, S(s)], a_sb[:, S(s)])
                nc.vector.tensor_mul(qp_sb[:, S(s)], qe_sb[:, S(s)],
                                     ra_sb[:, S(s)])

            # intra-chunk quadratic term + P transposes
            gm_sb, pn_sb = {}, {}
            for c in range(NCH):
                gt_ps = pw.tile([CH, SEQ * CH], F32, tag="w")
                for s in range(SEQ):
                    nc.tensor.matmul(gt_ps[:, CH * s:CH * (s + 1)],
                                     pt_sb[:, tsl(s, c)], qp_sb[:, tsl(s, c)],
                                     start=True, stop=True)
                gm_sb[c] = lp.tile([CH, SEQ * CH], BF, tag="gm", name=f"gm{c}")
                nc.vector.tensor_mul(gm_sb[c][:], gt_ps[:], mask4_sb[:])

                if c < NCH - 1:
                    tr_ps = pw.tile([CH, SEQ * CH], BF, tag="w")
                    for s in range(SEQ):
                        nc.tensor.transpose(tr_ps[:, CH * s:CH * (s + 1)],
                                            pt_sb[:, tsl(s, c)], ident_sb[:])
                    pn_sb[c] = lp.tile([CH, SEQ * CH], BF, tag="pn", name=f"pn{c}")
                    nc.scalar.copy(pn_sb[c][:], tr_ps[:])

            s_tiles = []
            s_prev = None
            for c in range(NCH - 1):
                ds_ps = pw.tile([LR, SEQ * D_K], F32, tag="w", name=f"dsp{c}")
                for s in range(SEQ):
                    nc.tensor.matmul(ds_ps[:, D_K * s:D_K * (s + 1)],
                                     pn_sb[c][:, CH * s:CH * (s + 1)],
                                     v_sb[:, vsl(s, c)],
                                     start=True, stop=True)
                s_new = sp.tile([LR, SEQ * D_K], BF, tag=f"state{c}",
                                name=f"state{c}")
                if s_prev is None:
                    nc.scalar.copy(s_new[:], ds_ps[:])
                else:
                    nc.vector.tensor_add(s_new[:], ds_ps[:], s_prev[:])
                s_tiles.append(s_new)
                s_prev = s_new

            for c in range(NCH):
                out_ps = pw.tile([D_K, SEQ * CH], F32, tag="w")
                for s in range(SEQ):
                    nc.tensor.matmul(out_ps[:, CH * s:CH * (s + 1)],
                                     v_sb[:, vsl(s, c)],
                                     gm_sb[c][:, CH * s:CH * (s + 1)],
                                     start=True, stop=(c == 0))
                    if c > 0:
                        nc.tensor.matmul(out_ps[:, CH * s:CH * (s + 1)],
                                         s_tiles[c - 1][:, D_K * s:D_K * (s + 1)],
                                         qp_sb[:, tsl(s, c)],
                                         start=False, stop=True)
                out_sb = lp.tile([D_K, SEQ * CH], BF, tag="osb")
                nc.scalar.copy(out_sb[:], out_ps[:])
                nc.sync.dma_start(
                    out_d[:, :, CH * c:CH * (c + 1)].rearrange("s d t -> d s t"),
                    out_sb[:].rearrange("d (s t) -> d s t", s=SEQ))

    nc.compile()
    return nc


def _host_prep(Khf, Vhf, Qhf, planes_T, protos_T):
    """Fold + transpose + quantize inputs; build per-core in_maps."""
    Khf = np.asarray(Khf, dtype=np.float32)
    Vhf = np.asarray(Vhf, dtype=np.float32)
    Qhf = np.asarray(Qhf, dtype=np.float32)
    planes_T = np.asarray(planes_T, dtype=np.float32)
    protos_T = np.asarray(protos_T, dtype=np.float32)
    scale = np.sqrt(np.float32(D_K))

    def fold(x):
        return np.transpose(x, (0, 1, 3, 2, 4)).reshape(N_TOTAL, T_LEN, D_K)

    K2, Q2, V2 = fold(Khf), fold(Qhf), fold(Vhf)
    KT = np.ascontiguousarray(np.transpose(K2, (0, 2, 1))).astype(BF16)  # [N, dk, T]
    QT = np.ascontiguousarray(np.transpose(Q2, (0, 2, 1))).astype(BF16)
    V4 = V2.reshape(N_TOTAL, NCH, CH, D_K)

    w4 = np.zeros((128, LR), dtype=np.float32)
    wblk = np.zeros((LK, LR), dtype=np.float32)
    for l in range(L_TABLES):
        wblk[l * K_BITS:(l + 1) * K_BITS, l * R_CORNERS:(l + 1) * R_CORNERS] = \
            protos_T / scale
    for s in range(4):
        w4[32 * s:32 * s + 32, :] = wblk
    pw = np.zeros((128, LR + LK), dtype=BF16)
    pw[:, 0:LR] = w4.astype(BF16)
    pw[0:D_K, LR:LR + LK] = planes_T.astype(BF16)

    in_maps = []
    for core in range(NCORES):
        ns = slice(SEQ * core, SEQ * (core + 1))
        ktc = np.ascontiguousarray(KT[ns]).reshape(SEQ, D_K, T_LEN)
        qtc = np.ascontiguousarray(QT[ns]).reshape(SEQ, D_K, T_LEN)
        vc = np.ascontiguousarray(
            np.transpose(V4[ns], (2, 0, 1, 3))).astype(BF16)  # [128, seq, ch, dk]
        in_maps.append({
            "kt": np.ascontiguousarray(np.transpose(ktc, (1, 0, 2))).reshape(D_K, SEQ * T_LEN),
            "qt": np.ascontiguousarray(np.transpose(qtc, (1, 0, 2))).reshape(D_K, SEQ * T_LEN),
            "v": vc.reshape(CH, SEQ * NCH * D_K),
            "pw": pw,
        })
    return in_maps


def kernel(Khf, Vhf, Qhf, planes_T, protos_T, _results_hook=None):
    if "nc" not in _CACHE:
        _CACHE["nc"] = _build_module()
    nc = _CACHE["nc"]
    in_maps = _host_prep(Khf, Vhf, Qhf, planes_T, protos_T)
    res = run_bass_kernel_spmd(nc, in_maps, list(range(NCORES)))
    if _results_hook is not None:
        _results_hook(res)
    out = np.empty((N_TOTAL, T_LEN, D_K), dtype=np.float32)
    for core in range(NCORES):
        out_t = res.results[core]["out_t"].astype(np.float32)  # [SEQ, dk, T]
        out[SEQ * core:SEQ * (core + 1)] = np.transpose(out_t, (0, 2, 1))
    return np.ascontiguousarray(
        out.reshape(M_ENS, B_SZ, H_HEADS, T_LEN, D_K).transpose(0, 1, 3, 2, 4))

